# revision 2
# baseline (speedup 1.0000x reference)
"""Trainium2 Bass kernel for nn_BitwiseTasNet (encoder + 32 linear residual
blocks + sigmoid mask + transposed-conv decoder).

Restructuring (all folding host-side, exact in fp32):
  - eval-mode BatchNorms fold into GEMM weights / per-channel affine applied
    at PSUM eviction (ScalarE activation with per-partition scale+bias).
  - per-block additive constants propagate forward into the next block's
    eviction bias; the total lands in the final sigmoid's bias vector.
  - dilated depthwise 3-tap conv = 3 VectorE ops on a zero-haloed SBUF tile
    (tensor_scalar_mul + 2x scalar_tensor_tensor with per-channel taps).
  - encoder = host im2col (stride view) + GEMM.
  - decoder = 2 shifted GEMMs accumulating in PSUM (overlap-add on PE).

Device dataflow per block:
  GEMM1 (bf16) -> PSUM -> ACT evict w/ bn2 affine -> t (bf16, zero halo)
  -> 3 DVE tap ops -> v (bf16) -> GEMM2 (bf16) -> PSUM
  -> DVE f32 residual add into h -> GpSimd cast h -> bf16 for next GEMM1.

Sharding: data-parallel over batch N=4 on 4 cores (v1).
"""
import sys
import numpy as np
import ml_dtypes

sys.path.insert(0, "/opt/trn_rl_repo")

from concourse import bass, bacc, tile, mybir  # noqa: E402
from concourse.bass_utils import run_bass_kernel_spmd  # noqa: E402

# model dims (hardcoded per contract)
N, CIN, T = 4, 1, 8000
C, D, K = 256, 512, 3
FK, FS = 20, 10
REPEATS, BLOCKS = 4, 8
NB = REPEATS * BLOCKS
EPS = 1e-5
L = 803
PAD = 128              # t-tile halo (max dilation)
TW = L + 2 * PAD       # t-tile width
CHUNKS = [(0, 512), (512, L)]   # psum-bank-aligned matmul free-dim chunks

F32 = mybir.dt.float32
BF16 = mybir.dt.bfloat16
bf16 = ml_dtypes.bfloat16
AF = mybir.ActivationFunctionType
ALU = mybir.AluOpType


# ----------------------------------------------------------------- host math
def fold_params(inp):
    p = {k: np.asarray(v, dtype=np.float64) for k, v in inp.items()}
    a = {}
    for nm in ('bn1', 'bn2', 'bn3'):
        sc = p[nm + '_g'] / np.sqrt(p[nm + '_v'] + EPS)
        sh = p[nm + '_b'] - p[nm + '_m'] * sc
        a[nm] = (sc, sh)
    a1, c1 = a['bn1']; a2, c2 = a['bn2']; a3, c3 = a['bn3']
    W1p = p['w1'][:, :, :, 0] * a1[:, None, :]                 # [NB, D, C]
    beta1 = np.einsum('idc,ic->id', p['w1'][:, :, :, 0], c1)   # [NB, D]
    Wk = a3[:, None, :] * np.transpose(p['wd'][:, :, 0, :], (0, 2, 1))  # [NB,3,D]
    W2 = p['w2'][:, :, :, 0]                                   # [NB, C, D]
    beta2 = np.einsum('icd,id->ic', W2, c3)                    # [NB, C]
    s = np.zeros((NB + 1, C))
    for i in range(NB):
        s[i + 1] = s[i] + beta2[i]
    b2p = a2 * (beta1 + np.einsum('idc,ic->id', W1p, s[:NB])) + c2  # [NB, D]
    return dict(W1p=W1p, Wk=Wk, W2=W2, a2=a2, b2p=b2p, sig_bias=s[NB],
                Wenc=p['w_enc'][:, 0, :], Wdec=p['w_dec'][:, 0, :])


def im2col(x):
    xp = np.zeros((N, T + 2 * FK), dtype=np.float32)
    xp[:, FK:FK + T] = np.asarray(x, np.float32)[:, 0, :]
    idx = FS * np.arange(L)[None, :] + np.arange(FK)[:, None]  # [FK, L]
    return xp[:, idx]                                          # [N, FK, L]


def pack_host(f):
    """Pack folded params into DMA-friendly arrays."""
    # GEMM1 lhsT tiles: w1t[i, p, k*D + d] = W1p[i][d, k*128+p]
    w1t = np.zeros((NB, 128, 2 * D), np.float32)
    for k in range(2):
        w1t[:, :, k * D:(k + 1) * D] = np.transpose(
            f['W1p'][:, :, k * 128:(k + 1) * 128], (0, 2, 1))
    # GEMM2 lhsT tiles: w2t[i, p, k*C + m] = W2[i][m, k*128+p]
    w2t = np.zeros((NB, 128, 4 * C), np.float32)
    for k in range(4):
        w2t[:, :, k * C:(k + 1) * C] = np.transpose(
            f['W2'][:, :, k * 128:(k + 1) * 128], (0, 2, 1))
    # encoder lhsT: [FK, C]
    wenct = f['Wenc'].T.astype(np.float32)                     # [20, 256]
    # decoder lhsT packed: [128, k*20 + jj] = Wdec[k*128+p, jj]
    wdect = np.zeros((128, 40), np.float32)
    for k in range(2):
        wdect[:, k * 20:(k + 1) * 20] = f['Wdec'][k * 128:(k + 1) * 128, :]
    # per-partition vectors: [128, ncols] f32
    # per block i, D-row m (4 rows): cols = a2, b2p, Wk0, Wk1, Wk2
    nv = NB * 4 * 5 + 2
    vecs = np.zeros((128, nv), np.float32)
    for i in range(NB):
        for m in range(4):
            base = (i * 4 + m) * 5
            sl = slice(m * 128, (m + 1) * 128)
            vecs[:, base + 0] = f['a2'][i][sl]
            vecs[:, base + 1] = f['b2p'][i][sl]
            for kk in range(3):
                vecs[:, base + 2 + kk] = f['Wk'][i, kk][sl]
    for mc in range(2):
        vecs[:, NB * 4 * 5 + mc] = f['sig_bias'][mc * 128:(mc + 1) * 128]
    return dict(
        w1t=w1t.astype(bf16), w2t=w2t.astype(bf16),
        wenct=wenct.astype(bf16), wdect=wdect.astype(bf16), vecs=vecs)


def vcol(i, m, kind):
    off = {'a2': 0, 'b2': 1, 'W0': 2, 'W1': 3, 'W2': 4}[kind]
    return (i * 4 + m) * 5 + off


# -------------------------------------------------------------- device build
def build_nc(n_cores=4, n_blocks=NB):
    nc = bacc.Bacc("TRN2", target_bir_lowering=False, debug=False,
                   num_devices=n_cores)
    xcol_d = nc.dram_tensor("xcol", [FK, L], BF16, kind="ExternalInput")
    w1_d = nc.dram_tensor("w1t", [NB, 128, 2 * D], BF16, kind="ExternalInput")
    w2_d = nc.dram_tensor("w2t", [NB, 128, 4 * C], BF16, kind="ExternalInput")
    wenc_d = nc.dram_tensor("wenct", [FK, C], BF16, kind="ExternalInput")
    wdec_d = nc.dram_tensor("wdect", [128, 40], BF16, kind="ExternalInput")
    vecs_d = nc.dram_tensor("vecs", [128, NB * 4 * 5 + 2], F32,
                            kind="ExternalInput")
    out_d = nc.dram_tensor("out", [10, 800], F32, kind="ExternalOutput")

    with tile.TileContext(nc) as tc:
        with (
            tc.tile_pool(name="fix", bufs=1) as fix,
            tc.tile_pool(name="w1p", bufs=3) as w1pool,
            tc.tile_pool(name="w2p", bufs=3) as w2pool,
            tc.tile_pool(name="pg", bufs=2, space="PSUM") as pgp,
            tc.tile_pool(name="pr", bufs=2, space="PSUM") as prp,
        ):
            # persistent tiles
            vecs = fix.tile([128, NB * 4 * 5 + 2], F32, tag="vecs")
            xcol = fix.tile([FK, L], BF16, tag="xcol")
            wenc = fix.tile([FK, C], BF16, tag="wenc")
            wdec = fix.tile([128, 40], BF16, tag="wdec")
            hf = [fix.tile([128, L], F32, tag=f"hf{m}", name=f"hf{m}") for m in range(2)]
            hb = [fix.tile([128, L], BF16, tag=f"hb{m}", name=f"hb{m}") for m in range(2)]
            xe = [fix.tile([128, L], F32, tag=f"xe{m}", name=f"xe{m}") for m in range(2)]
            tt = [fix.tile([128, TW], BF16, tag=f"t{m}", name=f"t{m}") for m in range(4)]
            vv = [fix.tile([128, L], BF16, tag=f"v{m}", name=f"v{m}") for m in range(4)]
            yy = [fix.tile([128, L], BF16, tag=f"y{m}", name=f"y{m}") for m in range(2)]
            outsb = fix.tile([10, 800], F32, tag="outsb")

            nc.sync.dma_start(out=vecs[:], in_=vecs_d.ap())
            nc.sync.dma_start(out=xcol[:], in_=xcol_d.ap())
            nc.sync.dma_start(out=wenc[:], in_=wenc_d.ap())
            nc.sync.dma_start(out=wdec[:], in_=wdec_d.ap())

            # zero the t halos once (never written again)
            for m in range(4):
                nc.gpsimd.memset(tt[m][:, 0:PAD], 0.0)
                nc.gpsimd.memset(tt[m][:, PAD + L:TW], 0.0)

            # ---- encoder: xe = Wenc @ xcol ----
            for mc in range(2):
                pe = pgp.tile([128, L], F32, tag="pg")
                for (c0, c1) in CHUNKS:
                    nc.tensor.matmul(
                        pe[:, c0:c1], wenc[:, mc * 128:(mc + 1) * 128],
                        xcol[:, c0:c1], start=True, stop=True)
                nc.scalar.copy(hf[mc][:], pe[:])
                nc.vector.tensor_copy(xe[mc][:], hf[mc][:])
                nc.gpsimd.tensor_copy(hb[mc][:], hf[mc][:])

            # ---- residual blocks ----
            for i in range(n_blocks):
                d = 2 ** (i % BLOCKS)
                w1 = w1pool.tile([128, 2 * D], BF16, tag="w1")
                w2 = w2pool.tile([128, 4 * C], BF16, tag="w2")
                nc.sync.dma_start(out=w1[:], in_=w1_d.ap()[i])
                nc.sync.dma_start(out=w2[:], in_=w2_d.ap()[i])

                # GEMM1: g[m*128:(m+1)*128, :] , K over C (2 tiles)
                pg = []
                for m in range(4):
                    ps = pgp.tile([128, L], F32, tag="pg")
                    pg.append(ps)
                    for (c0, c1) in CHUNKS:
                        for k in range(2):
                            nc.tensor.matmul(
                                ps[:, c0:c1],
                                w1[:, k * D + m * 128: k * D + (m + 1) * 128],
                                hb[k][:, c0:c1],
                                start=(k == 0), stop=(k == 1))
                    # evict with bn2 affine -> bf16 t (zero-haloed)
                    nc.scalar.activation(
                        tt[m][:, PAD:PAD + L], ps[:],
                        AF.Identity,
                        bias=vecs[:, vcol(i, m, 'b2'):vcol(i, m, 'b2') + 1],
                        scale=vecs[:, vcol(i, m, 'a2'):vcol(i, m, 'a2') + 1])

                # taps: v = W0*t[l-d] + W1*t[l] + W2*t[l+d]
                for m in range(4):
                    c = [vecs[:, vcol(i, m, w):vcol(i, m, w) + 1]
                         for w in ('W0', 'W1', 'W2')]
                    nc.vector.tensor_scalar_mul(
                        vv[m][:], tt[m][:, PAD - d:PAD - d + L], c[0])
                    nc.vector.scalar_tensor_tensor(
                        vv[m][:], tt[m][:, PAD:PAD + L], c[1], vv[m][:],
                        ALU.mult, ALU.add)
                    nc.vector.scalar_tensor_tensor(
                        vv[m][:], tt[m][:, PAD + d:PAD + d + L], c[2], vv[m][:],
                        ALU.mult, ALU.add)

                # GEMM2: r[mc*128:...] , K over D (4 tiles) + residual
                for mc in range(2):
                    ps = prp.tile([128, L], F32, tag="pr")
                    for (c0, c1) in CHUNKS:
                        for k in range(4):
                            nc.tensor.matmul(
                                ps[:, c0:c1],
                                w2[:, k * C + mc * 128: k * C + (mc + 1) * 128],
                                vv[k][:, c0:c1],
                                start=(k == 0), stop=(k == 3))
                    nc.vector.tensor_add(hf[mc][:], hf[mc][:], ps[:])
                    nc.gpsimd.tensor_copy(hb[mc][:], hf[mc][:])

            # ---- mask + decoder ----
            sb = NB * 4 * 5
            for mc in range(2):
                mask = vv[mc]  # reuse a bf16 [128, L] tile
                nc.scalar.activation(mask[:], hf[mc][:], AF.Sigmoid,
                                     bias=vecs[:, sb + mc:sb + mc + 1])
                nc.vector.tensor_mul(yy[mc][:], xe[mc][:], mask[:])

            po = prp.tile([10, 800], F32, tag="pr")
            for (c0, c1) in [(0, 512), (512, 800)]:
                for k in range(2):
                    # out2d[j,q] = sum_c wdec[c,j]*y[c,q+2] + wdec[c,j+10]*y[c,q+1]
                    nc.tensor.matmul(
                        po[:, c0:c1], wdec[:, k * 20:k * 20 + 10],
                        yy[k][:, c0 + 2:c1 + 2],
                        start=(k == 0), stop=False)
                    nc.tensor.matmul(
                        po[:, c0:c1], wdec[:, k * 20 + 10:k * 20 + 20],
                        yy[k][:, c0 + 1:c1 + 1],
                        start=False, stop=(k == 1))
            nc.scalar.copy(outsb[:], po[:])
            nc.sync.dma_start(out=out_d.ap(), in_=outsb[:])

    nc.compile()
    return nc


# ------------------------------------------------------------------- driver
_CACHE = {}


def _get_nc(n_cores, n_blocks):
    key = (n_cores, n_blocks)
    if key not in _CACHE:
        _CACHE[key] = build_nc(n_cores, n_blocks)
    return _CACHE[key]


def run(inputs, n_blocks=NB, trace=False):
    f = fold_params(inputs)
    pk = pack_host(f)
    xc = im2col(inputs['x']).astype(bf16)
    n_cores = 4
    nc = _get_nc(n_cores, n_blocks)
    in_maps = []
    for n in range(n_cores):
        in_maps.append(dict(
            xcol=xc[n], w1t=pk['w1t'], w2t=pk['w2t'],
            wenct=pk['wenct'], wdect=pk['wdect'], vecs=pk['vecs']))
    res = run_bass_kernel_spmd(nc, in_maps, list(range(n_cores)), trace=trace)
    out = np.zeros((N, CIN, T), np.float32)
    for n in range(n_cores):
        out[n, 0, :] = res.results[n]['out'].T.reshape(T)
    return out, res


def kernel(**inputs):
    out, _ = run(inputs)
    return out


# revision 3
# speedup vs baseline: 1.2919x; 1.2919x over previous
"""Trainium2 Bass kernel for nn_BitwiseTasNet (encoder + 32 linear residual
blocks + sigmoid mask + transposed-conv decoder).

Restructuring (all folding host-side, exact in fp32):
  - eval-mode BatchNorms fold into GEMM weights / per-channel affine applied
    at PSUM eviction (ScalarE activation with per-partition scale+bias).
  - per-block additive constants propagate forward into the next block's
    eviction bias; the total lands in the final sigmoid's bias vector.
  - dilated depthwise 3-tap conv on zero-haloed SBUF tiles, split across
    engines: D-rows 0-2 on VectorE (3x tensor_scalar_mul @4x + 2x add @2x),
    row 3 via 3 pre-scaled ScalarE evictions + 2 GpSimd adds.
  - bf16 residual stream (validated rel_l2 1.25e-2 vs reference).
  - encoder = host im2col (stride view) + GEMM.
  - decoder = 2 shifted GEMMs accumulating in PSUM (overlap-add on PE).

Sharding: data-parallel over batch N=4 on 4 cores (pair-collectives measured
at ~20us/shot on this stack — per-block cross-core comm is not viable).
"""
import sys
import numpy as np
import ml_dtypes

sys.path.insert(0, "/opt/trn_rl_repo")

from concourse import bass, bacc, tile, mybir  # noqa: E402
from concourse.bass_utils import run_bass_kernel_spmd  # noqa: E402

# model dims (hardcoded per contract)
N, CIN, T = 4, 1, 8000
C, D, K = 256, 512, 3
FK, FS = 20, 10
REPEATS, BLOCKS = 4, 8
NB = REPEATS * BLOCKS
EPS = 1e-5
L = 803
W4 = 804               # even op width for DVE 4x mode
PAD = 128              # t-tile halo (max dilation)
TW = PAD + W4 + PAD
CHUNKS = [(0, 512), (512, L)]   # psum-bank-aligned matmul free-dim chunks

F32 = mybir.dt.float32
BF16 = mybir.dt.bfloat16
bf16 = ml_dtypes.bfloat16
AF = mybir.ActivationFunctionType
ALU = mybir.AluOpType

NVC = 4 * 5 + 6        # vec columns per block


# ----------------------------------------------------------------- host math
def fold_params(inp):
    p = {k: np.asarray(v, dtype=np.float64) for k, v in inp.items()}
    a = {}
    for nm in ('bn1', 'bn2', 'bn3'):
        sc = p[nm + '_g'] / np.sqrt(p[nm + '_v'] + EPS)
        sh = p[nm + '_b'] - p[nm + '_m'] * sc
        a[nm] = (sc, sh)
    a1, c1 = a['bn1']; a2, c2 = a['bn2']; a3, c3 = a['bn3']
    W1p = p['w1'][:, :, :, 0] * a1[:, None, :]                 # [NB, D, C]
    beta1 = np.einsum('idc,ic->id', p['w1'][:, :, :, 0], c1)   # [NB, D]
    Wk = a3[:, None, :] * np.transpose(p['wd'][:, :, 0, :], (0, 2, 1))  # [NB,3,D]
    W2 = p['w2'][:, :, :, 0]                                   # [NB, C, D]
    beta2 = np.einsum('icd,id->ic', W2, c3)                    # [NB, C]
    s = np.zeros((NB + 1, C))
    for i in range(NB):
        s[i + 1] = s[i] + beta2[i]
    b2p = a2 * (beta1 + np.einsum('idc,ic->id', W1p, s[:NB])) + c2  # [NB, D]
    return dict(W1p=W1p, Wk=Wk, W2=W2, a2=a2, b2p=b2p, sig_bias=s[NB],
                Wenc=p['w_enc'][:, 0, :], Wdec=p['w_dec'][:, 0, :])


def im2col(x):
    xp = np.zeros((N, T + 2 * FK), dtype=np.float32)
    xp[:, FK:FK + T] = np.asarray(x, np.float32)[:, 0, :]
    idx = FS * np.arange(L)[None, :] + np.arange(FK)[:, None]  # [FK, L]
    return xp[:, idx]                                          # [N, FK, L]


def pack_host(f):
    """Pack folded params into DMA-friendly arrays."""
    w1t = np.zeros((NB, 128, 2 * D), np.float32)
    for k in range(2):
        w1t[:, :, k * D:(k + 1) * D] = np.transpose(
            f['W1p'][:, :, k * 128:(k + 1) * 128], (0, 2, 1))
    w2t = np.zeros((NB, 128, 4 * C), np.float32)
    for k in range(4):
        w2t[:, :, k * C:(k + 1) * C] = np.transpose(
            f['W2'][:, :, k * 128:(k + 1) * 128], (0, 2, 1))
    wenct = f['Wenc'].T.astype(np.float32)                     # [20, 256]
    wdect = np.zeros((128, 40), np.float32)
    for k in range(2):
        wdect[:, k * 20:(k + 1) * 20] = f['Wdec'][k * 128:(k + 1) * 128, :]
    # per-partition vectors: per block: 4x(a2,b2,W0,W1,W2) + row-3 E0..E2,F0..F2
    nv = NB * NVC + 2
    vecs = np.zeros((128, nv), np.float32)
    for i in range(NB):
        for m in range(4):
            base = i * NVC + m * 5
            sl = slice(m * 128, (m + 1) * 128)
            vecs[:, base + 0] = f['a2'][i][sl]
            vecs[:, base + 1] = f['b2p'][i][sl]
            for kk in range(3):
                vecs[:, base + 2 + kk] = f['Wk'][i, kk][sl]
        sl = slice(3 * 128, 4 * 128)
        for kk in range(3):
            vecs[:, i * NVC + 20 + kk] = (f['a2'][i] * f['Wk'][i, kk])[sl]
            vecs[:, i * NVC + 23 + kk] = (f['b2p'][i] * f['Wk'][i, kk])[sl]
    for mc in range(2):
        vecs[:, NB * NVC + mc] = f['sig_bias'][mc * 128:(mc + 1) * 128]
    return dict(
        w1t=w1t.astype(bf16), w2t=w2t.astype(bf16),
        wenct=wenct.astype(bf16), wdect=wdect.astype(bf16), vecs=vecs)


def vcol(i, m, kind):
    off = {'a2': 0, 'b2': 1, 'W0': 2, 'W1': 3, 'W2': 4}[kind]
    return i * NVC + m * 5 + off


def vcol3(i, kind):
    off = {'E0': 20, 'E1': 21, 'E2': 22, 'F0': 23, 'F1': 24, 'F2': 25}[kind]
    return i * NVC + off


# -------------------------------------------------------------- device build
def build_nc(n_cores=4, n_blocks=NB):
    nc = bacc.Bacc("TRN2", target_bir_lowering=False, debug=False,
                   num_devices=n_cores)
    xcol_d = nc.dram_tensor("xcol", [FK, L], BF16, kind="ExternalInput")
    w1_d = nc.dram_tensor("w1t", [NB, 128, 2 * D], BF16, kind="ExternalInput")
    w2_d = nc.dram_tensor("w2t", [NB, 128, 4 * C], BF16, kind="ExternalInput")
    wenc_d = nc.dram_tensor("wenct", [FK, C], BF16, kind="ExternalInput")
    wdec_d = nc.dram_tensor("wdect", [128, 40], BF16, kind="ExternalInput")
    vecs_d = nc.dram_tensor("vecs", [128, NB * NVC + 2], F32,
                            kind="ExternalInput")
    out_d = nc.dram_tensor("out", [10, 800], F32, kind="ExternalOutput")

    with tile.TileContext(nc) as tc:
        with (
            tc.tile_pool(name="fix", bufs=1) as fix,
            tc.tile_pool(name="w1p", bufs=3) as w1pool,
            tc.tile_pool(name="w2p", bufs=3) as w2pool,
            tc.tile_pool(name="pg", bufs=2, space="PSUM") as pgp,
            tc.tile_pool(name="pr", bufs=2, space="PSUM") as prp,
        ):
            vecs = fix.tile([128, NB * NVC + 2], F32, tag="vecs")
            xcol = fix.tile([FK, L], BF16, tag="xcol")
            wenc = fix.tile([FK, C], BF16, tag="wenc")
            wdec = fix.tile([128, 40], BF16, tag="wdec")
            hb = [fix.tile([128, W4], BF16, tag=f"hb{m}", name=f"hb{m}")
                  for m in range(2)]
            xe = [fix.tile([128, L], F32, tag=f"xe{m}", name=f"xe{m}")
                  for m in range(2)]
            # rows 0-2: plain t; row 3: three pre-scaled copies
            tt = [fix.tile([128, TW], BF16, tag=f"t{m}", name=f"t{m}")
                  for m in range(3)]
            t3 = [fix.tile([128, TW], BF16, tag=f"t3{k}", name=f"t3{k}")
                  for k in range(3)]
            vv = [fix.tile([128, W4], BF16, tag=f"v{m}", name=f"v{m}")
                  for m in range(4)]
            tmp = [fix.tile([128, W4], BF16, tag=f"tmp{m}", name=f"tmp{m}")
                   for m in range(2)]
            yy = [fix.tile([128, L], BF16, tag=f"y{m}", name=f"y{m}")
                  for m in range(2)]
            outsb = fix.tile([10, 800], F32, tag="outsb")

            nc.sync.dma_start(out=vecs[:], in_=vecs_d.ap())
            nc.sync.dma_start(out=xcol[:], in_=xcol_d.ap())
            nc.sync.dma_start(out=wenc[:], in_=wenc_d.ap())
            nc.sync.dma_start(out=wdec[:], in_=wdec_d.ap())

            # zero halos once (never written again)
            for t in tt + t3:
                nc.gpsimd.memset(t[:, 0:PAD], 0.0)
                nc.gpsimd.memset(t[:, PAD + L:TW], 0.0)

            # ---- encoder: h0 = xe = Wenc @ xcol ----
            for mc in range(2):
                pe = pgp.tile([128, L], F32, tag="pg", name="pe")
                for (c0, c1) in CHUNKS:
                    nc.tensor.matmul(
                        pe[:, c0:c1], wenc[:, mc * 128:(mc + 1) * 128],
                        xcol[:, c0:c1], start=True, stop=True)
                nc.scalar.copy(hb[mc][:, 0:L], pe[:])
                nc.vector.tensor_copy(xe[mc][:], pe[:])

            # ---- residual blocks ----
            for i in range(n_blocks):
                d = 2 ** (i % BLOCKS)
                w1 = w1pool.tile([128, 2 * D], BF16, tag="w1", name="w1")
                w2 = w2pool.tile([128, 4 * C], BF16, tag="w2", name="w2")
                nc.sync.dma_start(out=w1[:], in_=w1_d.ap()[i])
                nc.sync.dma_start(out=w2[:], in_=w2_d.ap()[i])

                # GEMM1 + eviction per D-row
                for m in range(4):
                    ps = pgp.tile([128, L], F32, tag="pg", name="ps")
                    for (c0, c1) in CHUNKS:
                        for k in range(2):
                            nc.tensor.matmul(
                                ps[:, c0:c1],
                                w1[:, k * D + m * 128: k * D + (m + 1) * 128],
                                hb[k][:, c0:c1],
                                start=(k == 0), stop=(k == 1))
                    if m < 3:
                        nc.scalar.activation(
                            tt[m][:, PAD:PAD + L], ps[:], AF.Identity,
                            bias=vecs[:, vcol(i, m, 'b2'):vcol(i, m, 'b2') + 1],
                            scale=vecs[:, vcol(i, m, 'a2'):vcol(i, m, 'a2') + 1])
                    else:
                        for kk in range(3):
                            e = vcol3(i, f'E{kk}'); f_ = vcol3(i, f'F{kk}')
                            nc.scalar.activation(
                                t3[kk][:, PAD:PAD + L], ps[:], AF.Identity,
                                bias=vecs[:, f_:f_ + 1], scale=vecs[:, e:e + 1])

                # taps rows 0-2 on DVE
                for m in range(3):
                    c = [vecs[:, vcol(i, m, w):vcol(i, m, w) + 1]
                         for w in ('W0', 'W1', 'W2')]
                    nc.vector.tensor_scalar_mul(
                        vv[m][:], tt[m][:, PAD - d:PAD - d + W4], c[0])
                    nc.vector.tensor_scalar_mul(
                        tmp[0][:], tt[m][:, PAD:PAD + W4], c[1])
                    nc.vector.tensor_add(vv[m][:], vv[m][:], tmp[0][:])
                    nc.vector.tensor_scalar_mul(
                        tmp[1][:], tt[m][:, PAD + d:PAD + d + W4], c[2])
                    nc.vector.tensor_add(vv[m][:], vv[m][:], tmp[1][:])
                # row 3 on GpSimd (pre-scaled evictions)
                nc.gpsimd.tensor_add(
                    vv[3][:], t3[0][:, PAD - d:PAD - d + W4],
                    t3[1][:, PAD:PAD + W4])
                nc.gpsimd.tensor_add(
                    vv[3][:], vv[3][:], t3[2][:, PAD + d:PAD + d + W4])

                # GEMM2 + residual add (bf16 stream)
                for mc in range(2):
                    ps = prp.tile([128, L], F32, tag="pr", name="psr")
                    for (c0, c1) in CHUNKS:
                        for k in range(4):
                            nc.tensor.matmul(
                                ps[:, c0:c1],
                                w2[:, k * C + mc * 128: k * C + (mc + 1) * 128],
                                vv[k][:, c0:c1],
                                start=(k == 0), stop=(k == 3))
                    nc.vector.tensor_add(hb[mc][:, 0:L], hb[mc][:, 0:L], ps[:])

            # ---- mask + decoder ----
            sb = NB * NVC
            for mc in range(2):
                mask = vv[mc]
                nc.scalar.activation(mask[:, 0:L], hb[mc][:, 0:L], AF.Sigmoid,
                                     bias=vecs[:, sb + mc:sb + mc + 1])
                nc.vector.tensor_mul(yy[mc][:], xe[mc][:], mask[:, 0:L])

            po = prp.tile([10, 800], F32, tag="pr", name="po")
            for (c0, c1) in [(0, 512), (512, 800)]:
                for k in range(2):
                    nc.tensor.matmul(
                        po[:, c0:c1], wdec[:, k * 20:k * 20 + 10],
                        yy[k][:, c0 + 2:c1 + 2], start=(k == 0), stop=False)
                    nc.tensor.matmul(
                        po[:, c0:c1], wdec[:, k * 20 + 10:k * 20 + 20],
                        yy[k][:, c0 + 1:c1 + 1], start=False, stop=(k == 1))
            nc.scalar.copy(outsb[:], po[:])
            nc.sync.dma_start(out=out_d.ap(), in_=outsb[:])

    nc.compile()
    return nc


# ------------------------------------------------------------------- driver
_CACHE = {}


def _get_nc(n_cores, n_blocks):
    key = (n_cores, n_blocks)
    if key not in _CACHE:
        _CACHE[key] = build_nc(n_cores, n_blocks)
    return _CACHE[key]


def run(inputs, n_blocks=NB, trace=False):
    f = fold_params(inputs)
    pk = pack_host(f)
    xc = im2col(inputs['x']).astype(bf16)
    n_cores = 4
    nc = _get_nc(n_cores, n_blocks)
    in_maps = []
    for n in range(n_cores):
        in_maps.append(dict(
            xcol=xc[n], w1t=pk['w1t'], w2t=pk['w2t'],
            wenct=pk['wenct'], wdect=pk['wdect'], vecs=pk['vecs']))
    res = run_bass_kernel_spmd(nc, in_maps, list(range(n_cores)), trace=trace)
    out = np.zeros((N, CIN, T), np.float32)
    for n in range(n_cores):
        out[n, 0, :] = res.results[n]['out'].T.reshape(T)
    return out, res


def kernel(**inputs):
    out, _ = run(inputs)
    return out
